# revision 9
# baseline (speedup 1.0000x reference)
"""Multi-head attention block (B=4, N=2048, D=768, H=12) on 8 TRN2 NeuronCores.

Sharding: core i -> batch b = i//2, query-row half qh = i%2 (1024 rows).
Each core computes all 12 heads for its (batch, q-half): qkv projection,
flash-style attention in S^T layout (keys on partitions, queries on free
axis), and the full output projection for its rows. No collectives: every
core produces complete output rows; host just concatenates.

Matmul inputs are bf16 (1 cyc/row on PE vs 4 for fp32); all accumulation is
fp32 in PSUM. Softmax skips the max-subtraction (scores are ~N(0,1); exp is
exact in fp32 for this range). Row-sums come from a ones-column appended to
V so the O-matmul accumulates them for free.
"""

import sys

sys.path.insert(0, "/opt/trn_rl_repo")

import numpy as np
import ml_dtypes

import concourse.bass as bass
import concourse.tile as tile
from concourse import bacc, mybir
from concourse.bass_utils import run_bass_kernel_spmd
from concourse.masks import make_identity

B, N, D, H = 4, 2048, 768, 12
HD = D // H  # 64
SCALE = HD**-0.5
NCORES = 8
QR = N // 2  # q rows per core
KT = D // 128  # 6 contraction tiles
NT = N // 128  # 16 key-row tiles
BF = mybir.dt.bfloat16
F32 = mybir.dt.float32

TRACE = False
LAST_EXEC_NS = None
_CACHED_NC = None


def _body(tc, xt, xtq, wqkv, wproj, biasb, y, qattn):
    nc = tc.nc
    with (
        tc.tile_pool(name="const", bufs=1) as cp,
        tc.tile_pool(name="persist", bufs=1) as pp,
    ):
        ones_bf = cp.tile([1, 128], BF)
        nc.vector.memset(ones_bf[:], 1.0)
        ones_f32 = cp.tile([1, 128], F32)
        nc.vector.memset(ones_f32[:], 1.0)
        ident = cp.tile([128, 128], F32)
        make_identity(nc, ident[:])
        bias_sb = cp.tile([1, D], BF)
        nc.sync.dma_start(bias_sb[:], biasb[:])

        xt_sb = [pp.tile([128, N], BF, tag=f"xt{k}", name=f"xt{k}") for k in range(KT)]
        xtq_sb = [pp.tile([128, QR], BF, tag=f"xtq{k}", name=f"xtq{k}") for k in range(KT)]
        w_sb = [pp.tile([128, 3 * D], BF, tag=f"w{k}", name=f"w{k}") for k in range(KT)]
        wp_sb = [pp.tile([128, D], BF, tag=f"wp{k}", name=f"wp{k}") for k in range(KT)]
        qT_sb = [pp.tile([128, QR], BF, tag=f"qT{k}", name=f"qT{k}") for k in range(KT)]
        kT_sb = [pp.tile([128, N], BF, tag=f"kT{k}", name=f"kT{k}") for k in range(KT)]
        v_sb = [pp.tile([128, H * (HD + 1)], BF, tag=f"v{r}", name=f"v{r}") for r in range(NT)]
        oT_sb = [pp.tile([128, QR], BF, tag=f"oT{k}", name=f"oT{k}") for k in range(KT)]
        qattn_sb = pp.tile([16, H * 128], F32, tag="qattn_sb")

        for k in range(KT):
            ks = slice(k * 128, (k + 1) * 128)
            nc.sync.dma_start(xt_sb[k][:], xt[ks, :])
            nc.sync.dma_start(xtq_sb[k][:], xtq[ks, :])
            nc.sync.dma_start(w_sb[k][:], wqkv[ks, :])
            nc.sync.dma_start(wp_sb[k][:], wproj[ks, :])

        # ---- Phase 1: qkv projection ----
        # q^T, k^T in [feature, row] layout; v in [row, feature] layout with a
        # ones column appended per head ([128, 13*65]) for softmax row-sums.
        with tc.tile_pool(name="qkvps", bufs=4, space="PSUM") as qp:
            for c in range(12):  # col-tiles: 0..5 -> q, 6..11 -> k
                is_q = c < 6
                rhs_tiles = xtq_sb if is_q else xt_sb
                nrows = QR if is_q else N
                dst = qT_sb[c] if is_q else kT_sb[c - 6]
                for n0 in range(0, nrows, 512):
                    ps = qp.tile([128, 512], F32, tag="qkv_ps")
                    for k in range(KT):
                        nc.tensor.matmul(
                            ps[:],
                            lhsT=w_sb[k][:, c * 128 : (c + 1) * 128],
                            rhs=rhs_tiles[k][:, n0 : n0 + 512],
                            start=(k == 0),
                            stop=(k == KT - 1),
                        )
                    nc.vector.tensor_copy(dst[:, n0 : n0 + 512], ps[:])
            for r in range(NT):
                hv = v_sb[r][:].rearrange("p (h c) -> p h c", c=HD + 1)
                nc.vector.memset(hv[:, :, HD : HD + 1], 1.0)
                for j, (c0, cw) in enumerate([(2 * D, 512), (2 * D + 512, 256)]):
                    ps = qp.tile([128, cw], F32, tag=f"v_ps{j}", name=f"v_ps{j}", bufs=2)
                    for k in range(KT):
                        nc.tensor.matmul(
                            ps[:],
                            lhsT=xt_sb[k][:, r * 128 : (r + 1) * 128],
                            rhs=w_sb[k][:, c0 : c0 + cw],
                            start=(k == 0),
                            stop=(k == KT - 1),
                        )
                    h0 = (c0 - 2 * D) // HD
                    nc.vector.tensor_copy(
                        hv[:, h0 : h0 + cw // HD, 0:HD],
                        ps[:].rearrange("p (h c) -> p h c", c=HD),
                    )

        # ---- Phase 2: attention per head (S^T layout: [k-rows, q-cols]) ----
        with (
            tc.tile_pool(name="stps", bufs=2, space="PSUM") as stp,  # 2x2 banks
            tc.tile_pool(name="ops", bufs=1, space="PSUM") as op_,  # 2x1 banks
            tc.tile_pool(name="miscps", bufs=2, space="PSUM") as mp,  # 2x1 banks
            tc.tile_pool(name="attnsb", bufs=3) as sb2,
        ):
            for h in range(H):
                t6 = h // 2
                po = (h % 2) * 64
                qT_h = qT_sb[t6][po : po + 64, :]  # [64, QR]
                o_ps = [op_.tile([HD + 1, 512], F32, tag=f"o{qc}", name=f"o{qc}") for qc in range(2)]
                stage = sb2.tile([128, NT], F32, tag="stage")
                for kt in range(NT):
                    st = stp.tile([128, QR], F32, tag="st")
                    lhsT = kT_sb[t6][po : po + 64, kt * 128 : (kt + 1) * 128]
                    for qc in range(2):
                        nc.tensor.matmul(
                            st[:, qc * 512 : (qc + 1) * 512],
                            lhsT=lhsT,
                            rhs=qT_h[:, qc * 512 : (qc + 1) * 512],
                            start=True,
                            stop=True,
                        )
                    pT = sb2.tile([128, QR], BF, tag="pT")
                    nc.scalar.activation(
                        pT[:], st[:], mybir.ActivationFunctionType.Exp, scale=SCALE
                    )
                    vh = v_sb[kt][:, h * (HD + 1) : (h + 1) * (HD + 1)]  # [128, 65]
                    for qc in range(2):
                        nc.tensor.matmul(
                            o_ps[qc][:],
                            lhsT=vh,
                            rhs=pT[:, qc * 512 : (qc + 1) * 512],
                            start=(kt == 0),
                            stop=(kt == NT - 1),
                        )
                    nc.vector.tensor_copy(stage[:, kt : kt + 1], pT[:, 0:1])
                # evacuate o_ps to SBUF fast (frees the accumulator banks for
                # the next head), then normalize from SBUF: oT = o / rowsum
                o_raw = sb2.tile([HD + 1, QR], F32, tag="o_raw")
                for qc in range(2):
                    nc.vector.tensor_copy(
                        o_raw[:, qc * 512 : (qc + 1) * 512], o_ps[qc][:]
                    )
                rowsum = sb2.tile([1, QR], F32, tag="rowsum")
                nc.vector.tensor_copy(rowsum[:], o_raw[HD : HD + 1, :])
                recip = sb2.tile([1, QR], F32, tag="recip")
                nc.vector.reciprocal_approx_fast(recip[:], rowsum[:])
                for qc in range(2):
                    bc = mp.tile([64, 512], F32, tag="misc")
                    nc.tensor.matmul(
                        bc[:],
                        lhsT=ones_f32[:1, :64],
                        rhs=recip[:1, qc * 512 : (qc + 1) * 512],
                        start=True,
                        stop=True,
                    )
                    nc.vector.tensor_mul(
                        oT_sb[t6][po : po + 64, qc * 512 : (qc + 1) * 512],
                        o_raw[0:HD, qc * 512 : (qc + 1) * 512],
                        bc[:],
                    )
                # q_attn: query row 0's softmax row (valid on qh==0 cores)
                bc128 = mp.tile([128, 1], F32, tag="misc")
                nc.tensor.matmul(
                    bc128[:], lhsT=ones_f32[:1, :128], rhs=recip[:1, 0:1],
                    start=True, stop=True,
                )
                bc128_sb = sb2.tile([128, 1], F32, tag="bc128sb")
                nc.vector.tensor_copy(bc128_sb[:], bc128[:])
                stage2 = sb2.tile([128, NT], F32, tag="stage2")
                nc.vector.tensor_scalar_mul(stage2[:], stage[:], bc128_sb[:])
                tp = mp.tile([NT, 128], F32, tag="misc")
                nc.tensor.transpose(tp[:], stage2[:], ident[:])
                nc.vector.tensor_copy(qattn_sb[:, h * 128 : (h + 1) * 128], tp[:])
            nc.sync.dma_start(
                qattn.rearrange("h (t c) -> t h c", c=128),
                qattn_sb[:].rearrange("p (h c) -> p h c", c=128),
            )

        # ---- Phase 3: output projection (+bias via ones row) ----
        with (
            tc.tile_pool(name="pjps", bufs=4, space="PSUM") as pj,
            tc.tile_pool(name="ysb", bufs=3) as yp,
        ):
            for r in range(QR // 128):
                ytile = yp.tile([128, D], F32, tag="y")
                for n0, nw in [(0, 512), (512, 256)]:
                    ps = pj.tile([128, nw], F32, tag=f"pj{n0}")
                    for k in range(KT):
                        nc.tensor.matmul(
                            ps[:],
                            lhsT=oT_sb[k][:, r * 128 : (r + 1) * 128],
                            rhs=wp_sb[k][:, n0 : n0 + nw],
                            start=(k == 0),
                            stop=False,
                        )
                    nc.tensor.matmul(
                        ps[:],
                        lhsT=ones_bf[:1, :128],
                        rhs=bias_sb[:1, n0 : n0 + nw],
                        start=False,
                        stop=True,
                    )
                    nc.vector.tensor_copy(ytile[:, n0 : n0 + nw], ps[:])
                nc.sync.dma_start(y[r * 128 : (r + 1) * 128, :], ytile[:])


def _build():
    nc = bacc.Bacc("TRN2", target_bir_lowering=False, debug=False, num_devices=NCORES)
    xt = nc.declare_dram_parameter("xt", [D, N], BF, isOutput=False)
    xtq = nc.declare_dram_parameter("xtq", [D, QR], BF, isOutput=False)
    wqkv = nc.declare_dram_parameter("wqkv", [D, 3 * D], BF, isOutput=False)
    wproj = nc.declare_dram_parameter("wproj", [D, D], BF, isOutput=False)
    biasb = nc.declare_dram_parameter("biasb", [1, D], BF, isOutput=False)
    y = nc.declare_dram_parameter("y", [QR, D], F32, isOutput=True)
    qattn = nc.declare_dram_parameter("qattn", [H, N], F32, isOutput=True)
    with tile.TileContext(nc) as tc:
        _body(tc, xt.ap(), xtq.ap(), wqkv.ap(), wproj.ap(), biasb.ap(), y.ap(), qattn.ap())
    nc.finalize()
    return nc


def kernel(x, w_qkv, w_proj, b_proj):
    global _CACHED_NC, LAST_EXEC_NS
    if _CACHED_NC is None:
        _CACHED_NC = _build()
    nc = _CACHED_NC

    bf16 = ml_dtypes.bfloat16
    x = np.asarray(x, np.float32)
    wqkv_bf = np.ascontiguousarray(np.asarray(w_qkv, np.float32).astype(bf16))
    wproj_bf = np.ascontiguousarray(np.asarray(w_proj, np.float32).astype(bf16))
    bias_bf = np.ascontiguousarray(
        np.asarray(b_proj, np.float32).astype(bf16).reshape(1, D)
    )
    in_maps = []
    for i in range(NCORES):
        b, qh = divmod(i, 2)
        xtb = np.ascontiguousarray(x[b].T.astype(bf16))
        in_maps.append(
            {
                "xt": xtb,
                "xtq": np.ascontiguousarray(xtb[:, qh * QR : (qh + 1) * QR]),
                "wqkv": wqkv_bf,
                "wproj": wproj_bf,
                "biasb": bias_bf,
            }
        )

    res = run_bass_kernel_spmd(nc, in_maps, core_ids=list(range(NCORES)), trace=TRACE)
    LAST_EXEC_NS = res.exec_time_ns

    out = np.empty((B, N, D), np.float32)
    q_attn = np.empty((B, H, N), np.float32)
    for i, r in enumerate(res.results):
        b, qh = divmod(i, 2)
        out[b, qh * QR : (qh + 1) * QR] = np.asarray(r["y"], np.float32)
        if qh == 0:
            q_attn[b] = np.asarray(r["qattn"], np.float32)
    return out, q_attn


# revision 10
# speedup vs baseline: 1.2833x; 1.2833x over previous
"""Multi-head attention block (B=4, N=2048, D=768, H=12) on 8 TRN2 NeuronCores.

Sharding: core i -> batch b = i//2, query-row half qh = i%2 (1024 rows).
Each core computes all 12 heads for its (batch, q-half): qkv projection,
flash-style attention in S^T layout (keys on partitions, queries on free
axis), and the full output projection for its rows. No collectives: every
core produces complete output rows; host just concatenates.

Matmul inputs are bf16 (1 cyc/row on PE vs 4 for fp32); all accumulation is
fp32 in PSUM. Softmax skips the max-subtraction (scores are ~N(0,1); exp is
exact in fp32 for this range). Row-sums come from a ones-column appended to
V so the O-matmul accumulates them for free.

The emission order interleaves qkv column-tile work into the ACT-bound
attention kt-loops so the PE stream stays dense (keeps the HAM clock at
2.4 GHz) while the Scalar engine streams the exps.
"""

import sys

sys.path.insert(0, "/opt/trn_rl_repo")

import numpy as np
import ml_dtypes

import concourse.bass as bass
import concourse.tile as tile
from concourse import bacc, mybir
from concourse.bass_utils import run_bass_kernel_spmd
from concourse.masks import make_identity

B, N, D, H = 4, 2048, 768, 12
HD = D // H  # 64
SCALE = HD**-0.5
NCORES = 8
QR = N // 2  # q rows per core
KT = D // 128  # 6 contraction tiles
NT = N // 128  # 16 key-row tiles
BF = mybir.dt.bfloat16
F32 = mybir.dt.float32

TRACE = False
LAST_EXEC_NS = None
_CACHED_NC = None


def _body(tc, xt, xtq, wqkv, wproj, biasb, y, qattn):
    nc = tc.nc
    with (
        tc.tile_pool(name="const", bufs=1) as cp,
        tc.tile_pool(name="persist", bufs=1) as pp,
        # all PSUM lives in one scope: mm 2 banks + st 4 banks + o 2 banks
        tc.tile_pool(name="mmps", bufs=2, space="PSUM") as mmp,
        tc.tile_pool(name="stps", bufs=2, space="PSUM") as stp,
        tc.tile_pool(name="ops", bufs=1, space="PSUM") as op_,
        tc.tile_pool(name="work", bufs=3) as sb2,
    ):
        ones_bf = cp.tile([1, 128], BF)
        nc.vector.memset(ones_bf[:], 1.0)
        ones_f32 = cp.tile([1, 128], F32)
        nc.vector.memset(ones_f32[:], 1.0)
        ident = cp.tile([128, 128], F32)
        make_identity(nc, ident[:])
        bias_sb = cp.tile([1, D], BF)
        nc.sync.dma_start(bias_sb[:], biasb[:])

        xt_sb = [pp.tile([128, N], BF, tag=f"xt{k}", name=f"xt{k}") for k in range(KT)]
        xtq_sb = [pp.tile([128, QR], BF, tag=f"xtq{k}", name=f"xtq{k}") for k in range(KT)]
        w_sb = [pp.tile([128, 3 * D], BF, tag=f"w{k}", name=f"w{k}") for k in range(KT)]
        wp_sb = [pp.tile([128, D], BF, tag=f"wp{k}", name=f"wp{k}") for k in range(KT)]
        qT_sb = [pp.tile([128, QR], BF, tag=f"qT{k}", name=f"qT{k}") for k in range(KT)]
        kT_sb = [pp.tile([128, N], BF, tag=f"kT{k}", name=f"kT{k}") for k in range(KT)]
        v_sb = [pp.tile([128, H * (HD + 1)], BF, tag=f"v{r}", name=f"v{r}") for r in range(NT)]
        oT_sb = [pp.tile([128, QR], BF, tag=f"oT{k}", name=f"oT{k}") for k in range(KT)]
        qattn_sb = pp.tile([16, H * 128], F32, tag="qattn_sb")

        for k in range(KT):
            ks = slice(k * 128, (k + 1) * 128)
            nc.sync.dma_start(xt_sb[k][:], xt[ks, :])
            nc.sync.dma_start(xtq_sb[k][:], xtq[ks, :])
            nc.sync.dma_start(w_sb[k][:], wqkv[ks, :])
            nc.sync.dma_start(wp_sb[k][:], wproj[ks, :])

        # ---- emit helpers ----

        def emit_v_tile(r):
            # v in [row, feature] layout, ones column appended per head
            hv = v_sb[r][:].rearrange("p (h c) -> p h c", c=HD + 1)
            nc.vector.memset(hv[:, :, HD : HD + 1], 1.0)
            for c0, cw in [(2 * D, 512), (2 * D + 512, 256)]:
                ps = mmp.tile([128, 512], F32, tag="mm", name="vps")
                for k in range(KT):
                    nc.tensor.matmul(
                        ps[:, :cw],
                        lhsT=xt_sb[k][:, r * 128 : (r + 1) * 128],
                        rhs=w_sb[k][:, c0 : c0 + cw],
                        start=(k == 0),
                        stop=(k == KT - 1),
                    )
                h0 = (c0 - 2 * D) // HD
                nc.vector.tensor_copy(
                    hv[:, h0 : h0 + cw // HD, 0:HD],
                    ps[:, :cw].rearrange("p (h c) -> p h c", c=HD),
                )

        def qk_groups(c):
            # generator of closures: one psum-group (6 matmuls + copy) each
            is_q = c < 6
            rhs_tiles = xtq_sb if is_q else xt_sb
            nrows = QR if is_q else N
            dst = qT_sb[c] if is_q else kT_sb[c - 6]
            for n0 in range(0, nrows, 512):
                def emit(n0=n0):
                    ps = mmp.tile([128, 512], F32, tag="mm", name="qkps")
                    for k in range(KT):
                        nc.tensor.matmul(
                            ps[:],
                            lhsT=w_sb[k][:, c * 128 : (c + 1) * 128],
                            rhs=rhs_tiles[k][:, n0 : n0 + 512],
                            start=(k == 0),
                            stop=(k == KT - 1),
                        )
                    nc.vector.tensor_copy(dst[:, n0 : n0 + 512], ps[:])
                yield emit

        def emit_head(h, filler):
            # flash attention for one head; calls next(filler) between kt
            # steps to keep the PE stream dense while ACT runs the exps.
            t6 = h // 2
            po = (h % 2) * 64
            qT_h = qT_sb[t6][po : po + 64, :]  # [64, QR]
            o_ps = [
                op_.tile([HD + 1, 512], F32, tag=f"o{qc}", name=f"o{qc}")
                for qc in range(2)
            ]
            stage = sb2.tile([128, NT], F32, tag="stage")
            for kt in range(NT):
                st = stp.tile([128, QR], F32, tag="st")
                lhsT = kT_sb[t6][po : po + 64, kt * 128 : (kt + 1) * 128]
                for qc in range(2):
                    nc.tensor.matmul(
                        st[:, qc * 512 : (qc + 1) * 512],
                        lhsT=lhsT,
                        rhs=qT_h[:, qc * 512 : (qc + 1) * 512],
                        start=True,
                        stop=True,
                    )
                pT = sb2.tile([128, QR], BF, tag="pT")
                nc.scalar.activation(
                    pT[:], st[:], mybir.ActivationFunctionType.Exp, scale=SCALE
                )
                vh = v_sb[kt][:, h * (HD + 1) : (h + 1) * (HD + 1)]  # [128, 65]
                for qc in range(2):
                    nc.tensor.matmul(
                        o_ps[qc][:],
                        lhsT=vh,
                        rhs=pT[:, qc * 512 : (qc + 1) * 512],
                        start=(kt == 0),
                        stop=(kt == NT - 1),
                    )
                nc.vector.tensor_copy(stage[:, kt : kt + 1], pT[:, 0:1])
                if kt % 3 == 2 and filler:
                    try:
                        next(filler)()
                    except StopIteration:
                        filler = None
            # evacuate o_ps fast (frees accumulator banks), normalize from SBUF
            o_raw = sb2.tile([HD + 1, QR], F32, tag="o_raw")
            for qc in range(2):
                nc.vector.tensor_copy(o_raw[:, qc * 512 : (qc + 1) * 512], o_ps[qc][:])
            rowsum = sb2.tile([1, QR], F32, tag="rowsum")
            nc.vector.tensor_copy(rowsum[:], o_raw[HD : HD + 1, :])
            recip = sb2.tile([1, QR], F32, tag="recip")
            nc.vector.reciprocal_approx_fast(recip[:], rowsum[:])
            for qc in range(2):
                bc = mmp.tile([64, 512], F32, tag="mm", name="bc")
                nc.tensor.matmul(
                    bc[:],
                    lhsT=ones_f32[:1, :64],
                    rhs=recip[:1, qc * 512 : (qc + 1) * 512],
                    start=True,
                    stop=True,
                )
                nc.vector.tensor_mul(
                    oT_sb[t6][po : po + 64, qc * 512 : (qc + 1) * 512],
                    o_raw[0:HD, qc * 512 : (qc + 1) * 512],
                    bc[:],
                )
            # q_attn: query row 0's softmax row (valid on qh==0 cores)
            bc128 = mmp.tile([128, 512], F32, tag="mm", name="bc128")
            nc.tensor.matmul(
                bc128[:, 0:1], lhsT=ones_f32[:1, :128], rhs=recip[:1, 0:1],
                start=True, stop=True,
            )
            bc128_sb = sb2.tile([128, 1], F32, tag="bc128sb")
            nc.vector.tensor_copy(bc128_sb[:], bc128[:, 0:1])
            stage2 = sb2.tile([128, NT], F32, tag="stage2")
            nc.vector.tensor_scalar_mul(stage2[:], stage[:], bc128_sb[:])
            tp = mmp.tile([128, 512], F32, tag="mm", name="tp")
            nc.tensor.transpose(tp[0:NT, 0:128], stage2[:], ident[:])
            nc.vector.tensor_copy(qattn_sb[:, h * 128 : (h + 1) * 128], tp[0:NT, 0:128])
            return filler

        # ---- emission: v tiles, first q/k pair, then heads with interleave ----
        for r in range(NT):
            emit_v_tile(r)
        for g in qk_groups(0):
            g()
        for g in qk_groups(6):
            g()
        for p in range(6):
            if p < 5:
                filler = iter(
                    [*qk_groups(p + 1), *qk_groups(7 + p)]
                )  # next pair's q/k column tiles (6 psum groups)
            else:
                filler = iter([])
            filler = emit_head(2 * p, filler)
            filler = emit_head(2 * p + 1, filler)
            while True:  # drain any remaining filler groups
                try:
                    next(filler)()
                except (StopIteration, TypeError):
                    break

        nc.sync.dma_start(
            qattn.rearrange("h (t c) -> t h c", c=128),
            qattn_sb[:].rearrange("p (h c) -> p h c", c=128),
        )

        # ---- output projection (+bias via ones row) ----
        for r in range(QR // 128):
            ytile = sb2.tile([128, D], F32, tag="y")
            for n0, nw in [(0, 512), (512, 256)]:
                ps = mmp.tile([128, 512], F32, tag="mm", name="pjps")
                for k in range(KT):
                    nc.tensor.matmul(
                        ps[:, :nw],
                        lhsT=oT_sb[k][:, r * 128 : (r + 1) * 128],
                        rhs=wp_sb[k][:, n0 : n0 + nw],
                        start=(k == 0),
                        stop=False,
                    )
                nc.tensor.matmul(
                    ps[:, :nw],
                    lhsT=ones_bf[:1, :128],
                    rhs=bias_sb[:1, n0 : n0 + nw],
                    start=False,
                    stop=True,
                )
                nc.vector.tensor_copy(ytile[:, n0 : n0 + nw], ps[:, :nw])
            nc.sync.dma_start(y[r * 128 : (r + 1) * 128, :], ytile[:])


def _build():
    nc = bacc.Bacc("TRN2", target_bir_lowering=False, debug=False, num_devices=NCORES)
    xt = nc.declare_dram_parameter("xt", [D, N], BF, isOutput=False)
    xtq = nc.declare_dram_parameter("xtq", [D, QR], BF, isOutput=False)
    wqkv = nc.declare_dram_parameter("wqkv", [D, 3 * D], BF, isOutput=False)
    wproj = nc.declare_dram_parameter("wproj", [D, D], BF, isOutput=False)
    biasb = nc.declare_dram_parameter("biasb", [1, D], BF, isOutput=False)
    y = nc.declare_dram_parameter("y", [QR, D], F32, isOutput=True)
    qattn = nc.declare_dram_parameter("qattn", [H, N], F32, isOutput=True)
    with tile.TileContext(nc) as tc:
        _body(tc, xt.ap(), xtq.ap(), wqkv.ap(), wproj.ap(), biasb.ap(), y.ap(), qattn.ap())
    nc.finalize()
    return nc


def kernel(x, w_qkv, w_proj, b_proj):
    global _CACHED_NC, LAST_EXEC_NS
    if _CACHED_NC is None:
        _CACHED_NC = _build()
    nc = _CACHED_NC

    bf16 = ml_dtypes.bfloat16
    x = np.asarray(x, np.float32)
    wqkv_bf = np.ascontiguousarray(np.asarray(w_qkv, np.float32).astype(bf16))
    wproj_bf = np.ascontiguousarray(np.asarray(w_proj, np.float32).astype(bf16))
    bias_bf = np.ascontiguousarray(
        np.asarray(b_proj, np.float32).astype(bf16).reshape(1, D)
    )
    in_maps = []
    for i in range(NCORES):
        b, qh = divmod(i, 2)
        xtb = np.ascontiguousarray(x[b].T.astype(bf16))
        in_maps.append(
            {
                "xt": xtb,
                "xtq": np.ascontiguousarray(xtb[:, qh * QR : (qh + 1) * QR]),
                "wqkv": wqkv_bf,
                "wproj": wproj_bf,
                "biasb": bias_bf,
            }
        )

    res = run_bass_kernel_spmd(nc, in_maps, core_ids=list(range(NCORES)), trace=TRACE)
    LAST_EXEC_NS = res.exec_time_ns

    out = np.empty((B, N, D), np.float32)
    q_attn = np.empty((B, H, N), np.float32)
    for i, r in enumerate(res.results):
        b, qh = divmod(i, 2)
        out[b, qh * QR : (qh + 1) * QR] = np.asarray(r["y"], np.float32)
        if qh == 0:
            q_attn[b] = np.asarray(r["qattn"], np.float32)
    return out, q_attn


# revision 15
# speedup vs baseline: 1.3093x; 1.0203x over previous
"""Multi-head attention block (B=4, N=2048, D=768, H=12) on 8 TRN2 NeuronCores.

Sharding: core i -> batch b = i//2, query-row half qh = i%2 (1024 rows).
Each core computes all 12 heads for its (batch, q-half): qkv projection,
flash-style attention in S^T layout (keys on partitions, queries on free
axis), and the full output projection for its rows. No collectives: every
core produces complete output rows; host just concatenates.

Matmul inputs are bf16 (1 cyc/row on PE vs 4 for fp32); all accumulation is
fp32 in PSUM. Softmax skips the max-subtraction (scores are ~N(0,1); exp is
exact in fp32 for this range). Row-sums come from a ones-column appended to
V so the O-matmul accumulates them for free.

The emission order interleaves qkv column-tile work into the ACT-bound
attention kt-loops so the PE stream stays dense (keeps the HAM clock at
2.4 GHz) while the Scalar engine streams the exps.
"""

import sys

sys.path.insert(0, "/opt/trn_rl_repo")

import numpy as np
import ml_dtypes

import concourse.bass as bass
import concourse.tile as tile
from concourse import bacc, mybir
from concourse.bass_utils import run_bass_kernel_spmd
from concourse.masks import make_identity

B, N, D, H = 4, 2048, 768, 12
HD = D // H  # 64
SCALE = HD**-0.5
NCORES = 8
QR = N // 2  # q rows per core
KT = D // 128  # 6 contraction tiles
NT = N // 128  # 16 key-row tiles
BF = mybir.dt.bfloat16
F32 = mybir.dt.float32

TRACE = False
LAST_EXEC_NS = None
_CACHED_NC = None


def _body(tc, xt, xtq, wqkv, wproj, biasb, y, qattn):
    nc = tc.nc
    with (
        tc.tile_pool(name="const", bufs=1) as cp,
        tc.tile_pool(name="persist", bufs=1) as pp,
        # all PSUM lives in one scope: mm 2 banks + st 4 banks + o 2 banks
        tc.tile_pool(name="mmps", bufs=2, space="PSUM") as mmp,
        tc.tile_pool(name="stps", bufs=2, space="PSUM") as stp,
        tc.tile_pool(name="ops", bufs=1, space="PSUM") as op_,
        tc.tile_pool(name="work", bufs=3) as sb2,
    ):
        ones_bf = cp.tile([1, 128], BF)
        nc.vector.memset(ones_bf[:], 1.0)
        ones_f32 = cp.tile([1, 128], F32)
        nc.vector.memset(ones_f32[:], 1.0)
        ident = cp.tile([128, 128], F32)
        make_identity(nc, ident[:])
        bias_sb = cp.tile([1, D], BF)
        nc.sync.dma_start(bias_sb[:], biasb[:])

        xt_sb = [pp.tile([128, N], BF, tag=f"xt{k}", name=f"xt{k}") for k in range(KT)]
        xtq_sb = [pp.tile([128, QR], BF, tag=f"xtq{k}", name=f"xtq{k}") for k in range(KT)]
        w_sb = [pp.tile([128, 3 * D], BF, tag=f"w{k}", name=f"w{k}") for k in range(KT)]
        wp_sb = [pp.tile([128, D], BF, tag=f"wp{k}", name=f"wp{k}") for k in range(KT)]
        qT_sb = [pp.tile([128, QR], BF, tag=f"qT{k}", name=f"qT{k}") for k in range(KT)]
        kT_sb = [pp.tile([128, N], BF, tag=f"kT{k}", name=f"kT{k}") for k in range(KT)]
        v_sb = [pp.tile([128, H * (HD + 1)], BF, tag=f"v{r}", name=f"v{r}") for r in range(NT)]
        oT_sb = [pp.tile([128, QR], BF, tag=f"oT{k}", name=f"oT{k}") for k in range(KT)]
        qattn_sb = pp.tile([16, H * 128], F32, tag="qattn_sb")

        for k in range(KT):
            ks = slice(k * 128, (k + 1) * 128)
            nc.sync.dma_start(xt_sb[k][:], xt[ks, :])
            nc.sync.dma_start(xtq_sb[k][:], xtq[ks, :])
            nc.sync.dma_start(w_sb[k][:], wqkv[ks, :])
            nc.sync.dma_start(wp_sb[k][:], wproj[ks, :])

        # ---- emit helpers ----

        def _drain(filler):
            while filler:
                try:
                    next(filler)()
                except StopIteration:
                    break

        def v_groups(r):
            # v in [row, feature] layout, ones column appended per head
            hv = v_sb[r][:].rearrange("p (h c) -> p h c", c=HD + 1)
            for gi, (c0, cw) in enumerate([(2 * D, 512), (2 * D + 512, 256)]):
                def emit(gi=gi, c0=c0, cw=cw):
                    if gi == 0:
                        nc.vector.memset(hv[:, :, HD : HD + 1], 1.0)
                    ps = mmp.tile([128, 512], F32, tag="mm", name="vps")
                    for k in range(KT):
                        nc.tensor.matmul(
                            ps[:, :cw],
                            lhsT=xt_sb[k][:, r * 128 : (r + 1) * 128],
                            rhs=w_sb[k][:, c0 : c0 + cw],
                            start=(k == 0),
                            stop=(k == KT - 1),
                        )
                    h0 = (c0 - 2 * D) // HD
                    nc.vector.tensor_copy(
                        hv[:, h0 : h0 + cw // HD, 0:HD],
                        ps[:, :cw].rearrange("p (h c) -> p h c", c=HD),
                    )
                yield emit

        def qk_groups(c):
            # generator of closures: one psum-group (6 matmuls + copy) each
            is_q = c < 6
            rhs_tiles = xtq_sb if is_q else xt_sb
            nrows = QR if is_q else N
            dst = qT_sb[c] if is_q else kT_sb[c - 6]
            for n0 in range(0, nrows, 512):
                def emit(n0=n0):
                    ps = mmp.tile([128, 512], F32, tag="mm", name="qkps")
                    for k in range(KT):
                        nc.tensor.matmul(
                            ps[:],
                            lhsT=w_sb[k][:, c * 128 : (c + 1) * 128],
                            rhs=rhs_tiles[k][:, n0 : n0 + 512],
                            start=(k == 0),
                            stop=(k == KT - 1),
                        )
                    nc.vector.tensor_copy(dst[:, n0 : n0 + 512], ps[:])
                yield emit

        def emit_head(h, filler, every=3):
            # flash attention for one head; calls next(filler) between kt
            # steps to keep the PE stream dense while ACT runs the exps.
            t6 = h // 2
            po = (h % 2) * 64
            qT_h = qT_sb[t6][po : po + 64, :]  # [64, QR]
            o_ps = [
                op_.tile([HD + 1, 512], F32, tag=f"o{qc}", name=f"o{qc}")
                for qc in range(2)
            ]
            stage = sb2.tile([128, NT], F32, tag="stage")
            for kt in range(NT):
                st = stp.tile([128, QR], F32, tag="st")
                lhsT = kT_sb[t6][po : po + 64, kt * 128 : (kt + 1) * 128]
                for qc in range(2):
                    nc.tensor.matmul(
                        st[:, qc * 512 : (qc + 1) * 512],
                        lhsT=lhsT,
                        rhs=qT_h[:, qc * 512 : (qc + 1) * 512],
                        start=True,
                        stop=True,
                    )
                pT = sb2.tile([128, QR], BF, tag="pT")
                nc.scalar.activation(
                    pT[:], st[:], mybir.ActivationFunctionType.Exp, scale=SCALE
                )
                vh = v_sb[kt][:, h * (HD + 1) : (h + 1) * (HD + 1)]  # [128, 65]
                for qc in range(2):
                    nc.tensor.matmul(
                        o_ps[qc][:],
                        lhsT=vh,
                        rhs=pT[:, qc * 512 : (qc + 1) * 512],
                        start=(kt == 0),
                        stop=(kt == NT - 1),
                    )
                nc.vector.tensor_copy(stage[:, kt : kt + 1], pT[:, 0:1])
                if kt % every == every - 1 and filler:
                    try:
                        next(filler)()
                    except StopIteration:
                        filler = None
            # evacuate o_ps fast (frees accumulator banks), normalize from SBUF
            o_raw = sb2.tile([HD + 1, QR], F32, tag="o_raw")
            for qc in range(2):
                nc.vector.tensor_copy(o_raw[:, qc * 512 : (qc + 1) * 512], o_ps[qc][:])
            rowsum = sb2.tile([1, QR], F32, tag="rowsum")
            nc.vector.tensor_copy(rowsum[:], o_raw[HD : HD + 1, :])
            recip = sb2.tile([1, QR], F32, tag="recip")
            nc.vector.reciprocal_approx_fast(recip[:], rowsum[:])
            for qc in range(2):
                bc = mmp.tile([64, 512], F32, tag="mm", name="bc")
                nc.tensor.matmul(
                    bc[:],
                    lhsT=ones_f32[:1, :64],
                    rhs=recip[:1, qc * 512 : (qc + 1) * 512],
                    start=True,
                    stop=True,
                )
                nc.vector.tensor_mul(
                    oT_sb[t6][po : po + 64, qc * 512 : (qc + 1) * 512],
                    o_raw[0:HD, qc * 512 : (qc + 1) * 512],
                    bc[:],
                )
            # q_attn: query row 0's softmax row (valid on qh==0 cores)
            bc128 = mmp.tile([128, 512], F32, tag="mm", name="bc128")
            nc.tensor.matmul(
                bc128[:, 0:1], lhsT=ones_f32[:1, :128], rhs=recip[:1, 0:1],
                start=True, stop=True,
            )
            bc128_sb = sb2.tile([128, 1], F32, tag="bc128sb")
            nc.vector.tensor_copy(bc128_sb[:], bc128[:, 0:1])
            stage2 = sb2.tile([128, NT], F32, tag="stage2")
            nc.vector.tensor_scalar_mul(stage2[:], stage[:], bc128_sb[:])
            tp = mmp.tile([128, 512], F32, tag="mm", name="tp")
            nc.tensor.transpose(tp[0:NT, 0:128], stage2[:], ident[:])
            nc.vector.tensor_copy(qattn_sb[:, h * 128 : (h + 1) * 128], tp[0:NT, 0:128])
            return filler

        # ---- emission: first q/k pair + early v tiles, then heads with
        # the remaining v tiles / next pair's q/k columns as PE filler ----
        for g in qk_groups(0):
            g()
        for g in qk_groups(6):
            g()
        for r in range(8):
            for g in v_groups(r):
                g()
        for p in range(6):
            if p == 0:
                # head 0 streams the remaining v tiles (1 group per kt keeps
                # v_sb[kt] ahead of the O-matmul consuming it)
                f0 = iter([g for r in range(8, NT) for g in v_groups(r)])
                f1 = iter([*qk_groups(1), *qk_groups(7)])
                f0 = emit_head(0, f0, every=1)
                _drain(f0)
                f1 = emit_head(1, f1, every=3)
                _drain(f1)
            else:
                if p < 5:
                    filler = iter([*qk_groups(p + 1), *qk_groups(7 + p)])
                else:
                    filler = iter([])
                filler = emit_head(2 * p, filler, every=3)
                filler = emit_head(2 * p + 1, filler, every=3)
                _drain(filler)

        nc.sync.dma_start(
            qattn.rearrange("h (t c) -> t h c", c=128),
            qattn_sb[:].rearrange("p (h c) -> p h c", c=128),
        )

        # ---- output projection (+bias via ones row) ----
        for r in range(QR // 128):
            ytile = sb2.tile([128, D], F32, tag="y")
            for n0, nw in [(0, 512), (512, 256)]:
                ps = mmp.tile([128, 512], F32, tag="mm", name="pjps")
                for k in range(KT):
                    nc.tensor.matmul(
                        ps[:, :nw],
                        lhsT=oT_sb[k][:, r * 128 : (r + 1) * 128],
                        rhs=wp_sb[k][:, n0 : n0 + nw],
                        start=(k == 0),
                        stop=False,
                    )
                nc.tensor.matmul(
                    ps[:, :nw],
                    lhsT=ones_bf[:1, :128],
                    rhs=bias_sb[:1, n0 : n0 + nw],
                    start=False,
                    stop=True,
                )
                nc.vector.tensor_copy(ytile[:, n0 : n0 + nw], ps[:, :nw])
            nc.sync.dma_start(y[r * 128 : (r + 1) * 128, :], ytile[:])


def _build():
    nc = bacc.Bacc("TRN2", target_bir_lowering=False, debug=False, num_devices=NCORES)
    xt = nc.declare_dram_parameter("xt", [D, N], BF, isOutput=False)
    xtq = nc.declare_dram_parameter("xtq", [D, QR], BF, isOutput=False)
    wqkv = nc.declare_dram_parameter("wqkv", [D, 3 * D], BF, isOutput=False)
    wproj = nc.declare_dram_parameter("wproj", [D, D], BF, isOutput=False)
    biasb = nc.declare_dram_parameter("biasb", [1, D], BF, isOutput=False)
    y = nc.declare_dram_parameter("y", [QR, D], F32, isOutput=True)
    qattn = nc.declare_dram_parameter("qattn", [H, N], F32, isOutput=True)
    with tile.TileContext(nc) as tc:
        _body(tc, xt.ap(), xtq.ap(), wqkv.ap(), wproj.ap(), biasb.ap(), y.ap(), qattn.ap())
    nc.finalize()
    return nc


def kernel(x, w_qkv, w_proj, b_proj):
    global _CACHED_NC, LAST_EXEC_NS
    if _CACHED_NC is None:
        _CACHED_NC = _build()
    nc = _CACHED_NC

    bf16 = ml_dtypes.bfloat16
    x = np.asarray(x, np.float32)
    wqkv_bf = np.ascontiguousarray(np.asarray(w_qkv, np.float32).astype(bf16))
    wproj_bf = np.ascontiguousarray(np.asarray(w_proj, np.float32).astype(bf16))
    bias_bf = np.ascontiguousarray(
        np.asarray(b_proj, np.float32).astype(bf16).reshape(1, D)
    )
    in_maps = []
    for i in range(NCORES):
        b, qh = divmod(i, 2)
        xtb = np.ascontiguousarray(x[b].T.astype(bf16))
        in_maps.append(
            {
                "xt": xtb,
                "xtq": np.ascontiguousarray(xtb[:, qh * QR : (qh + 1) * QR]),
                "wqkv": wqkv_bf,
                "wproj": wproj_bf,
                "biasb": bias_bf,
            }
        )

    res = run_bass_kernel_spmd(nc, in_maps, core_ids=list(range(NCORES)), trace=TRACE)
    LAST_EXEC_NS = res.exec_time_ns

    out = np.empty((B, N, D), np.float32)
    q_attn = np.empty((B, H, N), np.float32)
    for i, r in enumerate(res.results):
        b, qh = divmod(i, 2)
        out[b, qh * QR : (qh + 1) * QR] = np.asarray(r["y"], np.float32)
        if qh == 0:
            q_attn[b] = np.asarray(r["qattn"], np.float32)
    return out, q_attn
